# revision 4
# baseline (speedup 1.0000x reference)
"""FFTEmbedding kernel for Trainium2 (8 NeuronCores, SPMD data-parallel over B).

Math: per (b, t): out = rfft(x_pad[b, t:t+W]) projected by weight + bias.
Linear in x, so it collapses to a causal conv with M2[w, e] (256, 512):
    out[b, t, e] = sum_w x_pad[b, t+w] * M2[w, e] + bias[e]

v2 design (per core: 2 batch rows, weights replicated):
  * WEIGHT-STATIONARY orientation: out tile = [e_blk 128, t 512] in PSUM.
    lhsT = M2 block [w 128, e 128] (8 distinct tiles), rhs = Hankel slice
    [w 128, t 512].  Hank[p, c] = x_pad[b, p + c] (mega-Hankel SBUF image).
  * [e, t] layout enables SINGLE-PASS evacuation with the bias fused as a
    per-partition vector: ACT activation(Identity, bias=AP) and DVE
    tensor_scalar(add, AP) both do PSUM->SBUF + bias + fp16 cast in one op.
    Evacuations are paired [128, 1024] (2 banks, segs s/s+1) and split
    between DVE (eb 0,1) and ACT (eb 2,3) - each engine ~35-38us << PE 55us.
  * Loop: row-outer, then 8 seg-pairs of 1024 t, then 4 e-blocks. PSUM =
    4 x [128, 1024] tiles = all 8 banks, recycled per seg-pair.
  * Output DRAM layout is [b, e, t] (host transposes back): per (row, eb)
    the sup tile [128, 8192] fp16 DMAs out in contiguous 2048-col waves
    (4 KB runs/partition vs 1 KB in v1 - much better DMA efficiency).
  * Hankel build: chunk0/1 (t<1536) load direct from HBM (128 shifted
    reads). Chunk2 (t in [1536, 8192)) loads only partitions 0:32 from HBM
    (stage1), then 3 SBUF->SBUF DMA copies replicate to partitions 32:128
    with col shifts (stage2, SWDGE) - cuts the redundant HBM read ~4x.
  * PE warm-up: HAM clock gate needs ~3.4us of sustained PE activity; junk
    matmuls (vs memset tile) start right at user-program start so the real
    MM stream runs at the warm 2.4 GHz rate (~216 ns / N=512 MM).
  * Output stored fp16 ([b, e, t]); host transposes to [b, t, e] and
    upcasts to fp32. Measured end-to-end rel err ~4e-4.
"""

import os
import sys

import numpy as np

_TRN_REPO = "/opt/trn_rl_repo"
if _TRN_REPO not in sys.path:
    sys.path.insert(0, _TRN_REPO)

B, T, W_SIZE, EMB = 16, 8192, 256, 512
N_CORES = 8
B_PER = B // N_CORES          # 2 batch rows per core
PAD = W_SIZE - 1              # 255 leading zeros
XP_LEN = T + PAD + 1          # 8448 (one trailing pad elem)

# t-space chunks of the Hankel image per row; chunk j covers t in
# [OFF[j], OFF[j+1]).  Boundaries must be multiples of 512.
CHUNKS = [512, 1024, 6656]
OFF = [0, 512, 1536, 8192]
STAGED = [False, False, True]  # chunk2 built via stage1(32 parts)+3 copies

N_SEG = T // 512              # 16 segs of 512 t per row
N_SP = N_SEG // 2             # 8 seg-pairs of 1024 t

# out-DMA waves per (row, eb): col ranges (last wave small for short tail)
WAVES = [(0, 2048), (2048, 4096), (4096, 6144), (6144, 7168), (7168, 8192)]

TRACE = os.environ.get("KERNEL_TRACE", "0") == "1"
N_WARM = int(os.environ.get("KERNEL_WARM", "5"))
LAST_RESULT = None

_CACHE = {}


def _build_m2(weight: np.ndarray) -> np.ndarray:
    """(EMB, 258) projection -> (W, EMB) causal-conv matrix, in float64."""
    k = np.arange(W_SIZE // 2 + 1, dtype=np.float64)   # 129
    w = np.arange(W_SIZE, dtype=np.float64)            # 256
    ang = 2.0 * np.pi * np.outer(k, w) / W_SIZE        # (129, 256)
    f = np.concatenate([np.cos(ang), -np.sin(ang)], axis=0)  # (258, 256)
    m2 = (weight.astype(np.float64) @ f).T             # (256, EMB)
    return np.ascontiguousarray(m2, dtype=np.float64)


def _build_program():
    from concourse import bacc, mybir, tile
    from concourse.ap import AP

    f32 = mybir.dt.float32
    f16 = mybir.dt.float16
    add = mybir.AluOpType.add
    ident = mybir.ActivationFunctionType.Identity

    nc = bacc.Bacc(target_bir_lowering=False)
    xpad_h = nc.declare_dram_parameter("xpad", [B_PER, XP_LEN], f16, isOutput=False)
    # w2 packed on host: w2[p, eb*256 + h*128 + m] = M2[128h + p, 128eb + m]
    w2_h = nc.declare_dram_parameter("w2", [128, 2 * EMB], f16, isOutput=False)
    # bias4[p, eb] = bias[128eb + p]
    bias4_h = nc.declare_dram_parameter("bias4", [128, 4], f32, isOutput=False)
    out_h = nc.declare_dram_parameter("out", [B_PER, EMB, T], f16, isOutput=True)

    with tile.TileContext(nc) as tc:
        with (
            tc.tile_pool(name="hank", bufs=1) as hank_pool,
            tc.tile_pool(name="wpool", bufs=1) as w_pool,
            tc.tile_pool(name="cpool", bufs=1) as c_pool,
            tc.tile_pool(name="sup", bufs=1) as sup_pool,
            tc.tile_pool(name="psum", bufs=4, space="PSUM") as psum_pool,
        ):
            # ---- PE warm-up: junk matmuls with no input dependency ----
            junk = c_pool.tile([128, 512], f16, tag="junk")
            nc.vector.memset(junk[:, :], 0.0)
            ps_warm = psum_pool.tile([128, 2 * EMB], f32, name="ps_warm", tag="ps")
            for _ in range(N_WARM):
                nc.tensor.matmul(
                    ps_warm[:, 0:EMB], junk[:, 0:128], junk[:, :],
                    start=True, stop=True,
                )

            # ---- constants / weights ----
            w01 = w_pool.tile([128, 2 * EMB], f16, tag="w01")
            # first (eb=0) slice arrives first so MMs can start early
            nc.scalar.dma_start(w01[:, 0:256], w2_h[:, 0:256])
            nc.scalar.dma_start(w01[:, 256 : 2 * EMB], w2_h[:, 256 : 2 * EMB])
            bias4 = c_pool.tile([128, 4], f32, tag="bias4")
            nc.scalar.dma_start(bias4[:, :], bias4_h[:, :])

            def wslice(h, eb):
                lo = eb * 256 + h * 128
                return w01[:, lo : lo + 128]

            # ---- Hankel images (one per batch row) ----
            # tile width: len + 128 (h=1 reach); staged chunks +96 more so
            # stage2 copies read within the tile.
            hank = [[None] * len(CHUNKS) for _ in range(B_PER)]

            def make_chunk_tiles(b):
                for j, ln in enumerate(CHUNKS):
                    w = ln + 224 if STAGED[j] else ln + 128
                    hank[b][j] = hank_pool.tile(
                        [128, w], f16, tag=f"hk{j}_{b}", name=f"hk{j}_{b}"
                    )

            def load_chunk(b, j, eng_direct):
                t = hank[b][j]
                base = b * XP_LEN + OFF[j]
                if not STAGED[j]:
                    cols = CHUNKS[j] + 128
                    eng_direct.dma_start(
                        t[:, :cols], AP(xpad_h, base, [[1, 128], [1, cols]])
                    )
                else:
                    cols1 = CHUNKS[j] + 224
                    eng_direct.dma_start(
                        t[0:32, :cols1], AP(xpad_h, base, [[1, 32], [1, cols1]])
                    )
                    cols2 = CHUNKS[j] + 128
                    for m in (1, 2, 3):
                        nc.gpsimd.dma_start(
                            t[32 * m : 32 * m + 32, 0:cols2],
                            t[0:32, 32 * m : 32 * m + cols2],
                        )

            make_chunk_tiles(0)
            make_chunk_tiles(1)
            # critical path: row0 chunk0 first on the sync ring
            load_chunk(0, 0, nc.sync)
            load_chunk(0, 1, nc.sync)
            load_chunk(0, 2, nc.scalar)
            load_chunk(1, 0, nc.sync)
            load_chunk(1, 1, nc.sync)
            load_chunk(1, 2, nc.scalar)

            def rhs(b, t0, h):
                """Hankel slice [w 128, t 512] for seg at t0, K-half h."""
                for j in range(len(CHUNKS)):
                    if t0 < OFF[j + 1]:
                        c0 = t0 - OFF[j] + 128 * h
                        return hank[b][j][:, c0 : c0 + 512]
                raise AssertionError(t0)

            # ---- sup (output staging) tiles ----
            sup = [
                [
                    sup_pool.tile([128, T], f16, tag=f"sup{b}_{eb}", name=f"sup{b}_{eb}")
                    for eb in range(4)
                ]
                for b in range(B_PER)
            ]

            # ---- main loop ----
            for b in range(B_PER):
                for sp in range(N_SP):
                    t0 = 1024 * sp
                    for eb in range(4):
                        ps = psum_pool.tile(
                            [128, 2 * EMB], f32, name=f"ps_{b}_{sp}_{eb}", tag="ps"
                        )
                        for s in range(2):
                            pslice = ps[:, s * 512 : (s + 1) * 512]
                            nc.tensor.matmul(
                                pslice, wslice(0, eb), rhs(b, t0 + 512 * s, 0),
                                start=True, stop=False,
                            )
                            nc.tensor.matmul(
                                pslice, wslice(1, eb), rhs(b, t0 + 512 * s, 1),
                                start=False, stop=True,
                            )
                        dst = sup[b][eb][:, t0 : t0 + 1024]
                        bvec = bias4[:, eb : eb + 1]
                        if eb < 2:
                            nc.vector.tensor_scalar(dst, ps[:, :], bvec, None, add)
                        else:
                            nc.scalar.activation(dst, ps[:, :], ident, bias=bvec)
                    # out-DMA waves that completed with this seg-pair
                    t_end = t0 + 1024
                    for wi, (lo, hi) in enumerate(WAVES):
                        if hi == t_end:
                            last = b == B_PER - 1 and hi == T
                            for eb in range(4):
                                # split the final wave across two rings to
                                # shorten the issue tail
                                eng = nc.scalar if (last and eb >= 2) else nc.sync
                                eng.dma_start(
                                    out_h[b, eb * 128 : (eb + 1) * 128, lo:hi],
                                    sup[b][eb][:, lo:hi],
                                )

    nc.finalize()
    return nc


def _get_program():
    if "prog" not in _CACHE:
        _CACHE["prog"] = _build_program()
    return _CACHE["prog"]


def kernel(x: np.ndarray, weight: np.ndarray, bias: np.ndarray) -> np.ndarray:
    global LAST_RESULT
    from concourse.bass_utils import run_bass_kernel_spmd

    x = np.asarray(x, dtype=np.float32)
    weight = np.asarray(weight, dtype=np.float32)
    bias = np.asarray(bias, dtype=np.float32)

    m2 = _build_m2(weight)
    xpad = np.zeros((B, XP_LEN), dtype=np.float32)
    xpad[:, PAD : PAD + T] = x
    # w2[p, eb*256 + h*128 + m] = M2[128h + p, 128eb + m]
    w2_in = np.ascontiguousarray(
        m2.reshape(2, 128, 4, 128).transpose(1, 2, 0, 3).reshape(128, 2 * EMB)
    ).astype(np.float16)
    bias4 = np.ascontiguousarray(bias.reshape(4, 128).T).astype(np.float32)
    xpad16 = xpad.astype(np.float16)

    nc = _get_program()
    in_maps = [
        {
            "xpad": np.ascontiguousarray(xpad16[c * B_PER : (c + 1) * B_PER]),
            "w2": w2_in,
            "bias4": bias4,
        }
        for c in range(N_CORES)
    ]
    res = run_bass_kernel_spmd(nc, in_maps, list(range(N_CORES)), trace=TRACE)
    LAST_RESULT = res
    out_bet = np.concatenate(
        [res.results[c]["out"] for c in range(N_CORES)], axis=0
    )  # (B, EMB, T) fp16
    out = out_bet.transpose(0, 2, 1).astype(np.float32)
    return np.ascontiguousarray(out)


# revision 11
# speedup vs baseline: 1.0332x; 1.0332x over previous
"""FFTEmbedding kernel for Trainium2 (8 NeuronCores, SPMD data-parallel over B).

Math: per (b, t): out = rfft(x_pad[b, t:t+W]) projected by weight + bias.
Linear in x, so it collapses to a causal conv with M2[w, e] (256, 512):
    out[b, t, e] = sum_w x_pad[b, t+w] * M2[w, e] + bias[e]

v2 design (per core: 2 batch rows, weights replicated):
  * WEIGHT-STATIONARY orientation: out tile = [e_blk 128, t 512] in PSUM.
    lhsT = M2 block [w 128, e 128] (8 distinct tiles), rhs = Hankel slice
    [w 128, t 512].  Hank[p, c] = x_pad[b, p + c] (mega-Hankel SBUF image).
  * [e, t] layout enables SINGLE-PASS evacuation with the bias fused as a
    per-partition vector: ACT activation(Identity, bias=AP) and DVE
    tensor_scalar(add, AP) both do PSUM->SBUF + bias + fp16 cast in one op.
    Evacuations are paired [128, 1024] (2 banks, segs s/s+1) and split
    between DVE (eb 0,1) and ACT (eb 2,3) - each engine ~35-38us << PE 55us.
  * Loop: row-outer, then 8 seg-pairs of 1024 t, then 4 e-blocks. PSUM =
    4 x [128, 1024] tiles = all 8 banks, recycled per seg-pair.
  * Output DRAM layout is [b, e, t] (host transposes back): per (row, eb)
    the sup tile [128, 8192] fp16 DMAs out in contiguous 2048-col waves
    (4 KB runs/partition vs 1 KB in v1 - much better DMA efficiency).
  * Hankel build: chunk0/1 (t<1536) load direct from HBM (128 shifted
    reads). Chunk2 (t in [1536, 8192)) loads only partitions 0:32 from HBM
    (stage1), then 3 SBUF->SBUF DMA copies replicate to partitions 32:128
    with col shifts (stage2, SWDGE) - cuts the redundant HBM read ~4x.
  * PE warm-up: HAM clock gate needs ~3.4us of sustained PE activity; junk
    matmuls (vs memset tile) start right at user-program start so the real
    MM stream runs at the warm 2.4 GHz rate (~216 ns / N=512 MM).
  * Output stored fp16 ([b, e, t]); host transposes to [b, t, e] and
    upcasts to fp32. Measured end-to-end rel err ~4e-4.
"""

import os
import sys

import numpy as np

_TRN_REPO = "/opt/trn_rl_repo"
if _TRN_REPO not in sys.path:
    sys.path.insert(0, _TRN_REPO)

B, T, W_SIZE, EMB = 16, 8192, 256, 512
N_CORES = 8
B_PER = B // N_CORES          # 2 batch rows per core
PAD = W_SIZE - 1              # 255 leading zeros
XP_LEN = T + PAD + 1          # 8448 (one trailing pad elem)

# t-space chunks of the Hankel image per row; chunk j covers t in
# [OFF[j], OFF[j+1]).  Boundaries must be multiples of 512.
CHUNKS = [512, 1536, 6144]
OFF = [0, 512, 2048, 8192]
# staged = built via stage1 (partitions 0:32 from HBM) + 3 SBUF->SBUF copies.
# row0 chunk0/1 are latency-critical -> direct; row1 has ~30us slack -> all
# staged.  Copies run on HWDGE rings (SWDGE/gpsimd delivers ~10us late).
STAGED = {0: [False, False, True], 1: [True, True, True]}

N_SEG = T // 512              # 16 segs of 512 t per row
N_SP = N_SEG // 2             # 8 seg-pairs of 1024 t

# out-DMA waves per (row, eb): col ranges (last wave small for short tail)
WAVES = [(0, 2048), (2048, 4096), (4096, 6144), (6144, 7168), (7168, 8192)]

TRACE = os.environ.get("KERNEL_TRACE", "0") == "1"
N_WARM = int(os.environ.get("KERNEL_WARM", "9"))
LAST_RESULT = None

_CACHE = {}


def _build_m2(weight: np.ndarray) -> np.ndarray:
    """(EMB, 258) projection -> (W, EMB) causal-conv matrix, in float64."""
    k = np.arange(W_SIZE // 2 + 1, dtype=np.float64)   # 129
    w = np.arange(W_SIZE, dtype=np.float64)            # 256
    ang = 2.0 * np.pi * np.outer(k, w) / W_SIZE        # (129, 256)
    f = np.concatenate([np.cos(ang), -np.sin(ang)], axis=0)  # (258, 256)
    m2 = (weight.astype(np.float64) @ f).T             # (256, EMB)
    return np.ascontiguousarray(m2, dtype=np.float64)


def _build_program():
    from concourse import bacc, mybir, tile
    from concourse.ap import AP

    f32 = mybir.dt.float32
    f16 = mybir.dt.float16
    add = mybir.AluOpType.add
    ident = mybir.ActivationFunctionType.Identity

    nc = bacc.Bacc(target_bir_lowering=False)
    xpad_h = nc.declare_dram_parameter("xpad", [B_PER, XP_LEN], f16, isOutput=False)
    # w2 packed on host: w2[p, eb*256 + h*128 + m] = M2[128h + p, 128eb + m]
    w2_h = nc.declare_dram_parameter("w2", [128, 2 * EMB], f16, isOutput=False)
    # bias4[p, eb] = bias[128eb + p]
    bias4_h = nc.declare_dram_parameter("bias4", [128, 4], f32, isOutput=False)
    out_h = nc.declare_dram_parameter("out", [B_PER, EMB, T], f16, isOutput=True)

    with tile.TileContext(nc) as tc:
        with (
            tc.tile_pool(name="hank", bufs=1) as hank_pool,
            tc.tile_pool(name="wpool", bufs=1) as w_pool,
            tc.tile_pool(name="cpool", bufs=1) as c_pool,
            tc.tile_pool(name="sup", bufs=1) as sup_pool,
            tc.tile_pool(name="psum", bufs=4, space="PSUM") as psum_pool,
        ):
            # ---- PE warm-up: junk matmuls with no input dependency ----
            # memset on gpsimd (otherwise idle); DVE stays clear for evacs
            junk = c_pool.tile([128, 512], f16, tag="junk")
            nc.gpsimd.memset(junk[:, :], 0.0)
            ps_warm = psum_pool.tile([128, 2 * EMB], f32, name="ps_warm", tag="ps")
            for _ in range(N_WARM):
                nc.tensor.matmul(
                    ps_warm[:, 0:EMB], junk[:, 0:128], junk[:, :],
                    start=True, stop=True,
                )

            # ---- constants / weights ----
            w01 = w_pool.tile([128, 2 * EMB], f16, tag="w01")
            nc.scalar.dma_start(w01[:, :], w2_h[:, :])
            bias4 = c_pool.tile([128, 4], f32, tag="bias4")

            def wslice(h, eb):
                lo = eb * 256 + h * 128
                return w01[:, lo : lo + 128]

            # ---- Hankel images (one per batch row) ----
            # tile width: len + 128 (h=1 reach); staged chunks +96 more so
            # stage2 copies read within the tile.
            hank = [[None] * len(CHUNKS) for _ in range(B_PER)]

            def make_chunk_tiles(b):
                for j, ln in enumerate(CHUNKS):
                    w = ln + 224 if STAGED[b][j] else ln + 128
                    hank[b][j] = hank_pool.tile(
                        [128, w], f16, tag=f"hk{j}_{b}", name=f"hk{j}_{b}"
                    )

            def stage1(b, j, eng):
                t = hank[b][j]
                base = b * XP_LEN + OFF[j]
                if not STAGED[b][j]:
                    cols = CHUNKS[j] + 128
                    eng.dma_start(
                        t[:, :cols], AP(xpad_h, base, [[1, 128], [1, cols]])
                    )
                else:
                    cols1 = CHUNKS[j] + 224
                    eng.dma_start(
                        t[0:32, :cols1], AP(xpad_h, base, [[1, 32], [1, cols1]])
                    )

            def stage2(b, j, eng):
                t = hank[b][j]
                cols2 = CHUNKS[j] + 128
                for m in (1, 2, 3):
                    eng.dma_start(
                        t[32 * m : 32 * m + 32, 0:cols2],
                        t[0:32, 32 * m : 32 * m + cols2],
                    )

            make_chunk_tiles(0)
            make_chunk_tiles(1)
            # critical path first: row0 chunk0/1 direct on the sync ring,
            # stage1 loads on the scalar ring, stage2 copies on sync (HWDGE)
            stage1(0, 0, nc.sync)      # direct [128, 640]
            stage1(0, 2, nc.scalar)    # after w2
            stage1(0, 1, nc.sync)      # direct [128, 1664]
            nc.scalar.dma_start(bias4[:, :], bias4_h[:, :])
            stage1(1, 0, nc.scalar)
            stage1(1, 1, nc.scalar)
            stage1(1, 2, nc.scalar)
            stage2(0, 2, nc.sync)
            stage2(1, 0, nc.sync)
            stage2(1, 1, nc.sync)
            stage2(1, 2, nc.sync)

            def rhs(b, t0, h):
                """Hankel slice [w 128, t 512] for seg at t0, K-half h."""
                for j in range(len(CHUNKS)):
                    if t0 < OFF[j + 1]:
                        c0 = t0 - OFF[j] + 128 * h
                        return hank[b][j][:, c0 : c0 + 512]
                raise AssertionError(t0)

            # ---- sup (output staging) tiles ----
            sup = [
                [
                    sup_pool.tile([128, T], f16, tag=f"sup{b}_{eb}", name=f"sup{b}_{eb}")
                    for eb in range(4)
                ]
                for b in range(B_PER)
            ]

            # ---- main loop ----
            for b in range(B_PER):
                for sp in range(N_SP):
                    t0 = 1024 * sp
                    # the kernel's final seg-pair gets fine-grained (per-seg)
                    # evacuation + immediate per-eb out-DMA to shorten the tail
                    last_sp = b == B_PER - 1 and sp == N_SP - 1
                    for eb in range(4):
                        ps = psum_pool.tile(
                            [128, 2 * EMB], f32, name=f"ps_{b}_{sp}_{eb}", tag="ps"
                        )
                        bvec = bias4[:, eb : eb + 1]
                        for s in range(2):
                            pslice = ps[:, s * 512 : (s + 1) * 512]
                            nc.tensor.matmul(
                                pslice, wslice(0, eb), rhs(b, t0 + 512 * s, 0),
                                start=True, stop=False,
                            )
                            nc.tensor.matmul(
                                pslice, wslice(1, eb), rhs(b, t0 + 512 * s, 1),
                                start=False, stop=True,
                            )
                            if last_sp:
                                dst = sup[b][eb][:, t0 + 512 * s : t0 + 512 * (s + 1)]
                                if eb < 2:
                                    nc.scalar.activation(dst, pslice, ident, bias=bvec)
                                else:
                                    nc.vector.tensor_scalar(dst, pslice, bvec, None, add)
                        if last_sp:
                            eng = nc.scalar if eb < 2 else nc.sync
                            eng.dma_start(
                                out_h[b, eb * 128 : (eb + 1) * 128, t0:T],
                                sup[b][eb][:, t0:T],
                            )
                        else:
                            dst = sup[b][eb][:, t0 : t0 + 1024]
                            if eb < 2:
                                nc.vector.tensor_scalar(dst, ps[:, :], bvec, None, add)
                            else:
                                nc.scalar.activation(dst, ps[:, :], ident, bias=bvec)
                    # out-DMA waves that completed with this seg-pair
                    t_end = t0 + 1024
                    for lo, hi in WAVES:
                        if hi == t_end and not (last_sp and hi == T):
                            for eb in range(4):
                                nc.sync.dma_start(
                                    out_h[b, eb * 128 : (eb + 1) * 128, lo:hi],
                                    sup[b][eb][:, lo:hi],
                                )

    nc.finalize()
    return nc


def _get_program():
    if "prog" not in _CACHE:
        _CACHE["prog"] = _build_program()
    return _CACHE["prog"]


def kernel(x: np.ndarray, weight: np.ndarray, bias: np.ndarray) -> np.ndarray:
    global LAST_RESULT
    from concourse.bass_utils import run_bass_kernel_spmd

    x = np.asarray(x, dtype=np.float32)
    weight = np.asarray(weight, dtype=np.float32)
    bias = np.asarray(bias, dtype=np.float32)

    m2 = _build_m2(weight)
    xpad = np.zeros((B, XP_LEN), dtype=np.float32)
    xpad[:, PAD : PAD + T] = x
    # w2[p, eb*256 + h*128 + m] = M2[128h + p, 128eb + m]
    w2_in = np.ascontiguousarray(
        m2.reshape(2, 128, 4, 128).transpose(1, 2, 0, 3).reshape(128, 2 * EMB)
    ).astype(np.float16)
    bias4 = np.ascontiguousarray(bias.reshape(4, 128).T).astype(np.float32)
    xpad16 = xpad.astype(np.float16)

    nc = _get_program()
    in_maps = [
        {
            "xpad": np.ascontiguousarray(xpad16[c * B_PER : (c + 1) * B_PER]),
            "w2": w2_in,
            "bias4": bias4,
        }
        for c in range(N_CORES)
    ]
    res = run_bass_kernel_spmd(nc, in_maps, list(range(N_CORES)), trace=TRACE)
    LAST_RESULT = res
    out_bet = np.concatenate(
        [res.results[c]["out"] for c in range(N_CORES)], axis=0
    )  # (B, EMB, T) fp16
    out = out_bet.transpose(0, 2, 1).astype(np.float32)
    return np.ascontiguousarray(out)
